# revision 1
# baseline (speedup 1.0000x reference)
"""Cross-attention kernel for 8 Trainium2 NeuronCores.

Problem (hardcoded): B=2, NQ=NKV=2048, QDIM=KVDIM=1024, H=16, HD=64.

Sharding: tensor-parallel over heads — 2 heads per core. Each core computes
its heads' Q/K/V projections, scores, softmax and context for the full
sequence, then an AllToAll reshards context from head-split to token-split
so the output projection is fully local; core j returns output tokens
[j*512, (j+1)*512).

All matmuls run in bf16 (fp32 PSUM accumulation). Layout trick: inputs are
fed pre-transposed ([feature, token]) so every matmul operand already has
its contraction dim on partitions — the kernel contains zero on-device
transposes. scores are computed transposed ([k, q]) so the exp'd
probabilities feed the P@V matmul directly as the stationary operand, and a
ones-column appended to V yields the softmax denominator from the same
matmul (no partition-axis reduction needed).
"""

import numpy as np
import ml_dtypes

import concourse.bass as bass
import concourse.mybir as mybir
import concourse.tile as tile
from concourse import bacc
from concourse.bass_utils import run_bass_kernel_spmd

N_CORES = 8
B = 2
NQ = NKV = 2048
C = 1024          # model dim (QDIM=KVDIM=INNER)
H, HD = 16, 64
T = B * NQ        # 4096 flattened tokens
DL = 128          # local head dims per core (2 heads * 64)
TSH = T // N_CORES  # 512 output tokens per core
SCALE = HD ** -0.5

F32 = mybir.dt.float32
BF16 = mybir.dt.bfloat16

_NC_CACHE = None
_LAST_RESULTS = None


def _build(with_collective=True, reps=None, stop_after=None):
    nc = bacc.Bacc("TRN2", target_bir_lowering=False, debug=False,
                   num_devices=N_CORES)

    qT = nc.dram_tensor("qT", [C, T], BF16, kind="ExternalInput")
    kvT = nc.dram_tensor("kvT", [C, T], BF16, kind="ExternalInput")
    wq = nc.dram_tensor("wq", [C, DL], BF16, kind="ExternalInput")
    wk = nc.dram_tensor("wk", [C, DL], BF16, kind="ExternalInput")
    wv = nc.dram_tensor("wv", [C, DL], BF16, kind="ExternalInput")
    wo = nc.dram_tensor("wo", [C, C], BF16, kind="ExternalInput")
    bias = nc.dram_tensor("bias", [C], F32, kind="ExternalInput")
    out = nc.dram_tensor("out", [TSH, C], F32, kind="ExternalOutput")

    CC = C // 128   # 8 contraction chunks
    KT = NKV // 128  # 16 k-tiles per batch
    Exp = mybir.ActivationFunctionType.Exp

    with tile.TileContext(nc) as tc:
        with (
            tc.tile_pool(name="consts", bufs=1) as consts,
            tc.tile_pool(name="xt", bufs=3) as xt,
            tc.tile_pool(name="probs", bufs=8) as probs_p,
            tc.tile_pool(name="norm", bufs=2) as norm,
            tc.tile_pool(name="outp", bufs=2) as outp,
            tc.tile_pool(name="dram", bufs=1, space="DRAM") as dram,
        ):
            # ---- constants ----
            wq_sb = consts.tile([128, CC, DL], BF16)
            nc.sync.dma_start(out=wq_sb, in_=wq.ap().rearrange("(n p) d -> p n d", p=128))
            wk_sb = consts.tile([128, CC, DL], BF16)
            nc.sync.dma_start(out=wk_sb, in_=wk.ap().rearrange("(n p) d -> p n d", p=128))
            wv_sb = consts.tile([128, CC, DL], BF16)
            nc.sync.dma_start(out=wv_sb, in_=wv.ap().rearrange("(n p) d -> p n d", p=128))
            wo_sb = consts.tile([128, CC, C], BF16)
            nc.sync.dma_start(out=wo_sb, in_=wo.ap().rearrange("(n p) e -> p n e", p=128))
            bias_sb = consts.tile([128, C], F32)
            bias_bc = bass.AP(tensor=bias, offset=0, ap=[[0, 128], [1, C]])
            nc.gpsimd.dma_start(out=bias_sb[:], in_=bias_bc)

            # persistent activations
            Kd_sb = consts.tile([128, T], BF16)   # K^T: [d_local, token]
            Qd_sb = consts.tile([128, T], BF16)   # Q^T: [d_local, token]
            # V natural [token, d] in 32 tiles of [128, 130]:
            # cols 0:64 = head0, col 64 = ones, 65:129 = head1, col 129 = ones
            V_sb = consts.tile([128, T // 128, 130], BF16)
            nc.vector.memset(V_sb[:, :, 64:65], 1.0)
            nc.vector.memset(V_sb[:, :, 129:130], 1.0)

            qT_r = qT.ap().rearrange("(n p) t -> p n t", p=128)
            kvT_r = kvT.ap().rearrange("(n p) t -> p n t", p=128)

            def _body(_it=None):
                # One shared PSUM pool for the whole body: tag "pss" slots
                # ([128,1024] = 2 banks, bufs=3) host projection / scores /
                # out-proj psums; tag "psc" (1 bank, bufs=2) hosts the two
                # per-head context accumulators. No pool-close barriers
                # between phases, so projections for batch 1 overlap the
                # ACT-bound attention of batch 0.
                with tc.tile_pool(name="ps", bufs=2, space="PSUM") as ps:

                    def proj(tt_range):
                        for tt in tt_range:
                            t0 = tt * 512
                            kvt = xt.tile([128, CC, 512], BF16, tag="kvt", name="kvt")
                            nc.sync.dma_start(out=kvt, in_=kvT_r[:, :, t0:t0 + 512])
                            qt_ = xt.tile([128, CC, 512], BF16, tag="qt", name="qt")
                            nc.sync.dma_start(out=qt_, in_=qT_r[:, :, t0:t0 + 512])

                            psk = ps.tile([128, 512], F32, tag="pss", name="psk")
                            for cc in range(CC):
                                nc.tensor.matmul(psk, lhsT=wk_sb[:, cc, :],
                                                 rhs=kvt[:, cc, :],
                                                 start=(cc == 0), stop=(cc == CC - 1))
                            nc.vector.tensor_copy(out=Kd_sb[:, t0:t0 + 512], in_=psk)

                            psq = ps.tile([128, 512], F32, tag="pss", name="psq")
                            for cc in range(CC):
                                nc.tensor.matmul(psq, lhsT=wq_sb[:, cc, :],
                                                 rhs=qt_[:, cc, :],
                                                 start=(cc == 0), stop=(cc == CC - 1))
                            nc.vector.tensor_copy(out=Qd_sb[:, t0:t0 + 512], in_=psq)

                            for s4 in range(4):
                                psv = ps.tile([128, 128], F32, tag="pss", name="psv")
                                for cc in range(CC):
                                    nc.tensor.matmul(
                                        psv, lhsT=kvt[:, cc, s4 * 128:(s4 + 1) * 128],
                                        rhs=wv_sb[:, cc, :],
                                        start=(cc == 0), stop=(cc == CC - 1))
                                ti = tt * 4 + s4
                                # one strided copy fills both head blocks
                                # (cols 0:64 and 65:129), skipping the ones
                                # columns: out free pattern [2 (stride 65), 64]
                                vdst = V_sb[:, ti, 0:64]
                                vdst2 = bass.AP(
                                    tensor=vdst.tensor, offset=vdst.offset,
                                    ap=[vdst.ap[0], [65, 2], [1, 64]])
                                nc.vector.tensor_copy(
                                    out=vdst2,
                                    in_=psv[:].rearrange("p (g x) -> p g x", g=2))

                    def attn_group(b, qv, psc, kt_range):
                        # Software-pipelined emission: scores(kt+1) is placed
                        # BEFORE pv(kt) in the (in-order) PE stream, so the
                        # PE never stalls on exp(kt) before issuing the next
                        # scores pair -- keeps the scalar engine (exp, the
                        # bottleneck) fed back-to-back.
                        q0 = b * NQ + qv * 512

                        def scores(kt):
                            k0 = b * NKV + kt * 128
                            # both heads' transposed scores into one
                            # 2-bank tile -> a single wide exp
                            pair = ps.tile([128, 1024], F32, tag="pss", name="pair")
                            for h in range(2):
                                hs = slice(h * 64, (h + 1) * 64)
                                nc.tensor.matmul(
                                    pair[:, h * 512:(h + 1) * 512],
                                    lhsT=Kd_sb[hs, k0:k0 + 128],
                                    rhs=Qd_sb[hs, q0:q0 + 512],
                                    start=True, stop=True)
                            return pair

                        def pv(kt, pr):
                            vt = b * KT + kt
                            for h in range(2):
                                nc.tensor.matmul(
                                    psc[h],
                                    lhsT=V_sb[:, vt, h * 65:(h + 1) * 65],
                                    rhs=pr[:, h * 512:(h + 1) * 512],
                                    start=(kt == 0), stop=(kt == KT - 1))

                        kts = list(kt_range)
                        pair = scores(kts[0])
                        for n, kt in enumerate(kts):
                            pr = probs_p.tile([128, 1024], BF16, tag="probs",
                                              name="pr")
                            nc.scalar.activation(out=pr, in_=pair, func=Exp,
                                                 scale=SCALE)
                            if n + 1 < len(kts):
                                pair = scores(kts[n + 1])
                            pv(kt, pr)

                    def attn_norm(b, qv, psc, a2a_in):
                        q0 = b * NQ + qv * 512
                        j = q0 // TSH
                        for h in range(2):
                            recip = norm.tile([1, 512], F32, tag="recip", name="recip")
                            nc.vector.reciprocal(out=recip, in_=psc[h][64:65, :])
                            bc = norm.tile([64, 512], F32, tag="bc", name="bc")
                            nc.gpsimd.partition_broadcast(bc[:], recip[:])
                            ctxn = norm.tile([64, 512], BF16, tag="ctxn", name="ctxn")
                            nc.vector.tensor_mul(ctxn, psc[h][0:64, :], bc)
                            nc.sync.dma_start(out=a2a_in[j, h * 64:(h + 1) * 64, :],
                                              in_=ctxn)

                    def alloc_psc():
                        return [ps.tile([65, 512], F32, tag="psc", name=f"psc{_h}",
                                        bufs=4) for _h in range(2)]

                    a2a_in = dram.tile([N_CORES, DL, TSH], BF16)
                    a2a_out = dram.tile([N_CORES, DL, TSH], BF16)

                    # batch-0 lead-in: interleave each projection t-tile
                    # with a 4-kt slice of the first attention q-tile so the
                    # scalar engine (exp, the bottleneck) starts early.
                    proj(range(0, 1))
                    psc0 = alloc_psc()
                    for tt in range(1, 4):
                        attn_group(0, 0, psc0, range((tt - 1) * 4, tt * 4))
                        proj(range(tt, tt + 1))
                    attn_group(0, 0, psc0, range(12, 16))
                    attn_norm(0, 0, psc0, a2a_in)
                    if stop_after == "proj":
                        return
                    # batch-0 qv1-3 with batch-1 projections threaded in,
                    # then batch-1 lead-in interleaved the same way as batch 0
                    psc_ = alloc_psc()
                    attn_group(0, 1, psc_, range(KT))
                    attn_norm(0, 1, psc_, a2a_in)
                    proj(range(4, 5))
                    psc_ = alloc_psc()
                    attn_group(0, 2, psc_, range(KT))
                    attn_norm(0, 2, psc_, a2a_in)
                    proj(range(5, 6))
                    psc_ = alloc_psc()
                    attn_group(0, 3, psc_, range(KT))
                    attn_norm(0, 3, psc_, a2a_in)
                    psc1 = alloc_psc()
                    attn_group(1, 0, psc1, range(0, 4))
                    proj(range(6, 7))
                    attn_group(1, 0, psc1, range(4, 8))
                    proj(range(7, 8))
                    attn_group(1, 0, psc1, range(8, 16))
                    attn_norm(1, 0, psc1, a2a_in)
                    for qv in range(1, 4):
                        psc_ = alloc_psc()
                        attn_group(1, qv, psc_, range(KT))
                        attn_norm(1, qv, psc_, a2a_in)

                    if stop_after == "attn":
                        return
                    if with_collective:
                        nc.gpsimd.collective_compute(
                            "AllToAll", mybir.AluOpType.bypass,
                            replica_groups=[list(range(N_CORES))],
                            ins=[a2a_in.opt()], outs=[a2a_out.opt()])
                    else:
                        a2a_out = a2a_in  # timing-sim variant: skip collective

                    # ---- output projection (local tokens only) ----
                    ctxF = outp.tile([128, N_CORES, TSH], BF16)
                    for i in range(N_CORES):
                        nc.sync.dma_start(out=ctxF[:, i, :], in_=a2a_out[i])
                    for m in range(TSH // 128):
                        ob = outp.tile([128, C], F32, tag="ob", name="ob")
                        for half in range(2):
                            pso = ps.tile([128, 512], F32, tag="pss", name="pso")
                            for i in range(N_CORES):
                                nc.tensor.matmul(
                                    pso, lhsT=ctxF[:, i, m * 128:(m + 1) * 128],
                                    rhs=wo_sb[:, i, half * 512:(half + 1) * 512],
                                    start=(i == 0), stop=(i == N_CORES - 1))
                            nc.vector.tensor_add(ob[:, half * 512:(half + 1) * 512],
                                                 pso,
                                                 bias_sb[:, half * 512:(half + 1) * 512])
                        nc.sync.dma_start(out=out.ap()[m * 128:(m + 1) * 128, :], in_=ob)

            if reps is None:
                _body()
            else:
                with tc.For_i(0, reps, 1) as _it:
                    _body(_it)
    nc.compile()
    return nc


def _get_nc():
    global _NC_CACHE
    if _NC_CACHE is None:
        _NC_CACHE = _build()
    return _NC_CACHE


def prep_in_maps(query, key_value, w_q, w_kv, w_out, b_out):
    bf = ml_dtypes.bfloat16
    q2 = np.asarray(query, np.float32).reshape(T, C)
    kv2 = np.asarray(key_value, np.float32).reshape(T, C)
    qT = np.ascontiguousarray(q2.T).astype(bf)
    kvT = np.ascontiguousarray(kv2.T).astype(bf)
    wo = np.asarray(w_out, np.float32).astype(bf)
    bias = np.asarray(b_out, np.float32)

    in_maps = []
    for j in range(N_CORES):
        cs = slice(j * DL, (j + 1) * DL)
        in_maps.append({
            "qT": qT,
            "kvT": kvT,
            "wq": np.ascontiguousarray(np.asarray(w_q, np.float32)[:, cs]).astype(bf),
            "wk": np.ascontiguousarray(np.asarray(w_kv, np.float32)[:, cs]).astype(bf),
            "wv": np.ascontiguousarray(
                np.asarray(w_kv, np.float32)[:, C + j * DL: C + (j + 1) * DL]).astype(bf),
            "wo": wo,
            "bias": bias,
        })
    return in_maps


def kernel(query, key_value, w_q, w_kv, w_out, b_out):
    global _LAST_RESULTS
    in_maps = prep_in_maps(query, key_value, w_q, w_kv, w_out, b_out)
    nc = _get_nc()
    res = run_bass_kernel_spmd(nc, in_maps, core_ids=list(range(N_CORES)))
    _LAST_RESULTS = res
    full = np.concatenate([res.results[j]["out"] for j in range(N_CORES)], axis=0)
    return full.reshape(B, NQ, C)

